# revision 1
# baseline (speedup 1.0000x reference)
# DCNv2 (modulated deformable conv) Trainium2 Bass kernel.
#
# Sharding: pure data parallel over 8 cores; core = (batch, H-half), each
# core computes a (256, 32, 64) output slab from a zero-padded input slab.
#
# Per-core pipeline:
#   1. offset/mask 3x3 conv on the PE (bf16 matmuls, fp32 PSUM, fused
#      bias (+tap/pad constants) and sigmoid on the ACT engine)
#   2. sampling fields (py/px/floor/frac/bilinear weights incl. mask) on DVE
#   3. pair-row gather: SWDGE dma_gather from an HBM-resident transposed
#      copy of x (row l = [x[:,l], x[:,l+1]]), one gathered row per SBUF
#      partition (hw-on-partitions, sigma-wrapped order)
#   4. 4-corner weighted combine via scalar_tensor_tensor with per-partition
#      scalars -> colsT, PE-transposed to cols (c-on-partitions)
#   5. main contraction out[o,hw] = sum_{c,p} w[o,c,p] cols[c,p,hw] as
#      bf16 matmuls accumulating in PSUM; bias + un-sigma on the ACT copy.
import numpy as np
import ml_dtypes

import concourse.bass as bass
import concourse.mybir as mybir
from concourse import bacc
import concourse.tile as tile
from concourse import library_config
from concourse.masks import make_identity
from concourse import bass_utils

BF16 = ml_dtypes.bfloat16

B, C, H, W = 4, 256, 64, 64
O, K = 256, 3
KK = K * K
NCORES = 8
HH = H // 2            # 32 output rows per core
PAD = 5                # zero halo; requires |offset| < PAD - 1
HP, WP = 48, 80        # padded local input dims
L = HP * WP            # 3840 source locations
NHW = HH * W           # 2048 output positions per core
NT = NHW // 128        # 16 gather tiles
CG = C // 128
OG = O // 128
A = mybir.AluOpType
ACTF = mybir.ActivationFunctionType
FP32 = mybir.dt.float32
BF = mybir.dt.bfloat16
I16 = mybir.dt.int16

MAGIC = float(np.float32(2 ** 23))


def build_nc():
    nc = bacc.Bacc("TRN2", target_bir_lowering=False, num_devices=NCORES)

    x_cm_d = nc.dram_tensor("x_cm", [CG, 128, HP, WP], BF, kind="ExternalInput").ap()
    xTp_d = nc.dram_tensor("xTp", [L, 2 * C], BF, kind="ExternalInput").ap()
    w_om_d = nc.dram_tensor("w_om", [128, KK, CG, 73], BF, kind="ExternalInput").ap()
    b_om_d = nc.dram_tensor("b_om", [73, 1], FP32, kind="ExternalInput").ap()
    w_mm_d = nc.dram_tensor("w_mm", [128, KK, CG, OG, 128], BF,
                            kind="ExternalInput").ap()
    b_o_d = nc.dram_tensor("b_o", [128, OG, 1], FP32, kind="ExternalInput").ap()
    hio_d = nc.dram_tensor("hio", [KK, NHW], BF, kind="ExternalInput").ap()
    wio_d = nc.dram_tensor("wio", [KK, NHW], BF, kind="ExternalInput").ap()
    y_d = nc.dram_tensor("y", [OG, 128, NHW], FP32, kind="ExternalOutput").ap()

    with tile.TileContext(nc) as tc:
        with (
            tc.tile_pool(name="const", bufs=1) as const,
            tc.tile_pool(name="persist", bufs=1) as persist,
            tc.tile_pool(name="dramp", bufs=3, space="DRAM") as dramp,
            tc.tile_pool(name="ps_conv", bufs=1, space="PSUM") as ps_conv,
            tc.tile_pool(name="ps_ft", bufs=2, space="PSUM") as ps_ft,
            tc.tile_pool(name="ps_t", bufs=2, space="PSUM") as ps_t,
            tc.tile_pool(name="ps_m", bufs=2, space="PSUM") as ps_m,
        ):
            # ---- constants into SBUF ----
            w_om = const.tile([128, KK, CG, 73], BF)
            nc.sync.dma_start(w_om[:], w_om_d)
            w_mm = const.tile([128, KK, CG, OG, 128], BF)
            nc.sync.dma_start(w_mm[:], w_mm_d)
            b_om = const.tile([73, 1], FP32)
            nc.sync.dma_start(b_om[:], b_om_d)
            b_o = const.tile([128, OG, 1], FP32)
            nc.sync.dma_start(b_o[:], b_o_d)
            hio = const.tile([KK, NHW], BF)
            nc.sync.dma_start(hio[:], hio_d)
            wio = const.tile([KK, NHW], BF)
            nc.sync.dma_start(wio[:], wio_d)
            id32 = const.tile([32, 32], FP32)
            make_identity(nc, id32[:])
            idbf = const.tile([128, 128], BF)
            make_identity(nc, idbf[:])
            nc.gpsimd.load_library(library_config.mlp)

            idx16a = persist.tile([KK, NHW], I16)
            idx16b = persist.tile([KK, NHW], I16)

            fld_cm = tc.tile_pool(name="fld", bufs=1)
            fld = fld_cm.__enter__()
            x_sb = []
            for cg in range(CG):
                xt = fld.tile([128, HP, WP], BF, name=f"xsb{cg}")
                nc.sync.dma_start(xt[:], x_cm_d[cg])
                x_sb.append(xt)

            # ---- offset/mask conv ----
            # psum channel layout: [0:9] off_y, [32:41] off_x, [64:73] mask
            # (engine APs may only start at partitions 0/32/64/96); each is
            # copied to its own base-0 tile (TensorTensor requires equal
            # base partitions for SBUF operands)
            offy_s = fld.tile([KK, NHW], FP32)
            offx_s = fld.tile([KK, NHW], FP32)
            msk = fld.tile([KK, NHW], FP32)
            for blk in range(8):
                # matmul rhs must be one contiguous free dim: stream 4 full
                # padded rows (N=320) and discard the pad columns on copy-out
                ps = ps_conv.tile([73, 4 * WP], FP32, tag="psc")
                r0 = blk * 4
                n = 0
                for cg in range(CG):
                    for tap in range(KK):
                        ky, kx = tap // K, tap % K
                        rhs = x_sb[cg][:, r0 + 4 + ky, 4 + kx:]
                        rhs = bass.AP(tensor=rhs.tensor, offset=rhs.offset,
                                      ap=[rhs.ap[0], [1, 4 * WP]])
                        nc.tensor.matmul(ps[:], w_om[:, tap, cg], rhs,
                                         start=(n == 0), stop=(n == 2 * KK - 1))
                        n += 1
                sl = slice(blk * 4 * W, (blk + 1) * 4 * W)
                psv = [None, None, None]
                for i, base in enumerate((0, 32, 64)):
                    p4 = ps[base:base + 9].rearrange("c (r x) -> c r x", r=4)
                    psv[i] = p4[:, :, 0:W]
                nc.scalar.activation(offy_s[:, sl], psv[0], ACTF.Identity,
                                     bias=b_om[0:9])
                nc.scalar.activation(offx_s[:, sl], psv[1], ACTF.Identity,
                                     bias=b_om[32:41])
                nc.scalar.activation(msk[:, sl], psv[2], ACTF.Sigmoid,
                                     bias=b_om[64:73])

            # ---- sampling fields [KK, NHW] f32 ----
            py = fld.tile([KK, NHW], FP32, tag="t_pyx", bufs=2, name="py")
            px = fld.tile([KK, NHW], FP32, tag="t_pyx", bufs=2, name="px")
            nc.vector.tensor_tensor(py[:], offy_s[:], hio[:], A.add)
            nc.vector.tensor_tensor(px[:], offx_s[:], wio[:], A.add)

            def floor_clamp(dst, src, hi):
                # dst = clamp(floor(src), 0, hi) via magic-number round(src-0.5)
                t1 = fld.tile([KK, NHW], FP32, tag="fc1", name="fc1", bufs=1)
                nc.vector.tensor_scalar(t1[:], src[:], MAGIC - 0.5, None, A.add)
                nc.vector.tensor_scalar(t1[:], t1[:], MAGIC, None, A.subtract)
                nc.vector.tensor_scalar(dst[:], t1[:], 0.0, float(hi),
                                        A.max, A.min)

            y0 = fld.tile([KK, NHW], FP32)
            x0 = fld.tile([KK, NHW], FP32)
            floor_clamp(y0, py, HP - 2)
            floor_clamp(x0, px, WP - 2)
            fy = fld.tile([KK, NHW], FP32)
            fx = fld.tile([KK, NHW], FP32)
            nc.vector.tensor_tensor(fy[:], py[:], y0[:], A.subtract)
            nc.vector.tensor_tensor(fx[:], px[:], x0[:], A.subtract)

            u = fld.tile([KK, NHW], FP32, tag="t_pyx", bufs=2, name="u")
            gy = fld.tile([KK, NHW], FP32, tag="t_pyx", bufs=2, name="gy")
            nc.vector.tensor_tensor(u[:], fy[:], msk[:], A.mult)
            nc.vector.tensor_tensor(gy[:], msk[:], u[:], A.subtract)
            w01 = fld.tile([KK, NHW], FP32, tag="t_w", bufs=2, name="w01")
            w00 = fld.tile([KK, NHW], FP32, tag="t_w", bufs=2, name="w00")
            nc.vector.tensor_tensor(w01[:], gy[:], fx[:], A.mult)
            nc.vector.tensor_tensor(w00[:], gy[:], w01[:], A.subtract)

            idxf = fld.tile([KK, NHW], FP32, tag="idxf", name="idxf")
            nc.vector.scalar_tensor_tensor(idxf[:], y0[:], float(WP), x0[:],
                                           A.mult, A.add)
            nc.vector.tensor_copy(idx16a[:], idxf[:])
            nc.vector.tensor_scalar(idxf[:], idxf[:], float(WP), None, A.add)
            nc.vector.tensor_copy(idx16b[:], idxf[:])

            # sigma-reorder bilinear weight fields so per-tile transposes
            # read contiguous [9,128] slices
            w00s = persist.tile([KK, NHW], FP32)
            w01s = persist.tile([KK, NHW], FP32)
            w10s = persist.tile([KK, NHW], FP32)
            w11s = persist.tile([KK, NHW], FP32)

            def sigma_copy(wdst, wsrc):
                src = wsrc[:].rearrange("c (t p a) -> c t p a", t=NT,
                                        p=16).transpose([0, 1, 3, 2])
                nc.vector.tensor_copy(wdst[:], src)

            sigma_copy(w01s, w01)
            sigma_copy(w00s, w00)
            w11 = fld.tile([KK, NHW], FP32, tag="t_w", bufs=2, name="w11")
            w10 = fld.tile([KK, NHW], FP32, tag="t_w", bufs=2, name="w10")
            nc.vector.tensor_tensor(w11[:], u[:], fx[:], A.mult)
            nc.vector.tensor_tensor(w10[:], u[:], w11[:], A.subtract)
            sigma_copy(w11s, w11)
            sigma_copy(w10s, w10)

            fld_cm.__exit__(None, None, None)

            # ---- per-tile gather + combine + matmul ----
            ftp_cm = tc.tile_pool(name="ftp", bufs=3)
            ftp = ftp_cm.__enter__()
            qp_cm = tc.tile_pool(name="qp", bufs=3)
            qp = qp_cm.__enter__()
            accp_cm = tc.tile_pool(name="accp", bufs=2)
            accp = accp_cm.__enter__()
            colsTp_cm = tc.tile_pool(name="colsTp", bufs=4)
            colsTp = colsTp_cm.__enter__()
            colsp_cm = tc.tile_pool(name="colsp", bufs=2)
            colsp = colsp_cm.__enter__()
            wrapp_cm = tc.tile_pool(name="wrapp", bufs=3)
            wrapp = wrapp_cm.__enter__()
            outp_cm = tc.tile_pool(name="outp", bufs=2)
            outp = outp_cm.__enter__()
            corners = [w00s, w01s, w10s, w11s]
            cols_sb = None
            for t in range(NT):
                # bilinear corner weights, sigma-ordered on partitions
                cwT = ftp.tile([128, 4, KK], FP32, tag="cwT", name="cwT")
                for j, wf in enumerate(corners):
                    psf = ps_ft.tile([128, KK], FP32, tag="psf", name="psf")
                    nc.tensor.transpose(psf[:], wf[0:9, t * 128:(t + 1) * 128],
                                        id32[0:9, 0:9])
                    nc.scalar.activation(cwT[:, j], psf[:], ACTF.Identity)

                # idx -> DRAM bounce (wrapped [16,144] layout) -> replicated
                db = dramp.tile([16, 2 * KK, 8], I16, tag="db", name="db")
                for ih, idxh in enumerate((idx16a, idx16b)):
                    dst_ap = bass.AP(
                        tensor=db.tensor, offset=db.offset + ih * KK * 8,
                        ap=[[8, KK], [144, 16], [1, 8]],
                    )
                    src_w = idxh[:, t * 128:(t + 1) * 128].rearrange(
                        "b (p q) -> b p q", p=16)
                    nc.sync.dma_start(dst_ap, src_w)
                wrapped = wrapp.tile([128, 2 * KK * 8], I16, tag="wr", name="wr")
                rep_ap = bass.AP(
                    tensor=db.tensor, offset=db.offset,
                    ap=[[0, 8], [144, 16], [1, 144]],
                )
                nc.sync.dma_start(wrapped[:], rep_ap)

                # gather Q[128(sig hw), 18(yp,tap), 512(v0|v1 channels)]
                Q = qp.tile([128, 2 * KK, 2 * C], BF, tag="Q", name="Q")
                nc.gpsimd.dma_gather(
                    out_ap=Q[:], in_ap=xTp_d, idxs_ap=wrapped[:],
                    num_idxs=2 * KK * 128, num_idxs_reg=2 * KK * 128,
                    elem_size=2 * C, single_packet=False,
                )

                # weighted 4-corner combine -> colsT [128(sig hw), KK, C] bf16
                colsT = colsTp.tile([128, KK, C], BF, tag="colsT", name="colsT")
                acc = accp.tile([128, C], FP32, tag="acc", name="acc")
                for tap in range(KK):
                    nc.vector.tensor_scalar(
                        acc[:], Q[:, tap, 0:C], cwT[:, 0, tap:tap + 1], None,
                        A.mult)
                    nc.vector.scalar_tensor_tensor(
                        acc[:], Q[:, tap, C:2 * C], cwT[:, 1, tap:tap + 1],
                        acc[:], A.mult, A.add)
                    nc.vector.scalar_tensor_tensor(
                        acc[:], Q[:, KK + tap, 0:C], cwT[:, 2, tap:tap + 1],
                        acc[:], A.mult, A.add)
                    nc.vector.scalar_tensor_tensor(
                        colsT[:, tap], Q[:, KK + tap, C:2 * C],
                        cwT[:, 3, tap:tap + 1], acc[:], A.mult, A.add)

                # transpose to cols [128(c), KK, CG, 512(sig hw)] bf16
                if t % 4 == 0:
                    cols_sb = colsp.tile([128, KK, CG, 512], BF, tag="cols",
                                         name="cols")
                for tap in range(KK):
                    for cg in range(CG):
                        pst = ps_t.tile([128, 128], BF, tag="pst", name="pst")
                        nc.tensor.transpose(
                            pst[:], colsT[:, tap, cg * 128:(cg + 1) * 128],
                            idbf[:])
                        nc.scalar.activation(
                            cols_sb[:, tap, cg,
                                    (t % 4) * 128:(t % 4 + 1) * 128],
                            pst[:], ACTF.Identity)

                # main contraction per 4-tile group
                if t % 4 == 3:
                    g = t // 4
                    for og in range(OG):
                        psO = ps_m.tile([128, 512], FP32, tag="psO", name="psO")
                        n = 0
                        for cg in range(CG):
                            for tap in range(KK):
                                nc.tensor.matmul(
                                    psO[:], w_mm[:, tap, cg, og],
                                    cols_sb[:, tap, cg],
                                    start=(n == 0), stop=(n == 2 * KK - 1))
                                n += 1
                        out_t = outp.tile([128, 512], FP32, tag="out",
                                          name="out_t")
                        dst = out_t.rearrange(
                            "o (q p a) -> o q p a", q=4, p=16).transpose(
                            [0, 1, 3, 2])
                        nc.scalar.activation(dst, psO[:], ACTF.Identity,
                                             bias=b_o[:, og])
                        nc.sync.dma_start(y_d[og, :, g * 512:(g + 1) * 512],
                                          out_t[:])
            for p in (outp_cm, wrapp_cm, colsp_cm, colsTp_cm, accp_cm,
                      qp_cm, ftp_cm):
                p.__exit__(None, None, None)
    nc.compile()
    return nc


# ---------------- host side ----------------

def host_prep(x, w_off, b_off, w_mask, b_mask, w_dcn, b_dcn):
    """Build the 8 per-core input maps (pure layout prep / sharding)."""
    x = np.asarray(x, np.float32)
    w_off = np.asarray(w_off, np.float32)
    w_mask = np.asarray(w_mask, np.float32)
    b_off = np.asarray(b_off, np.float32)
    b_mask = np.asarray(b_mask, np.float32)
    w_dcn = np.asarray(w_dcn, np.float32)
    b_dcn = np.asarray(b_dcn, np.float32)

    wcat = np.zeros((73, C, K, K), np.float32)
    wcat[0:9] = w_off[0::2]
    wcat[32:41] = w_off[1::2]
    wcat[64:73] = w_mask
    w_om = np.zeros((128, KK, CG, 73), BF16)
    for tap in range(KK):
        ky, kx = tap // K, tap % K
        for cg in range(CG):
            w_om[:, tap, cg] = (
                wcat[:, cg * 128:(cg + 1) * 128, ky, kx].T.astype(BF16))

    ky_t = np.repeat(np.arange(K), K).astype(np.float32)
    kx_t = np.tile(np.arange(K), K).astype(np.float32)
    b_om = np.zeros((73, 1), np.float32)
    b_om[0:9, 0] = b_off[0::2] + ky_t - 1 + PAD
    b_om[32:41, 0] = b_off[1::2] + kx_t - 1 + PAD
    b_om[64:73, 0] = b_mask

    w_mm = np.zeros((128, KK, CG, OG, 128), BF16)
    for tap in range(KK):
        ky, kx = tap // K, tap % K
        for cg in range(CG):
            for og in range(OG):
                w_mm[:, tap, cg, og] = w_dcn[
                    og * 128:(og + 1) * 128, cg * 128:(cg + 1) * 128,
                    ky, kx].T.astype(BF16)
    b_o = b_dcn.reshape(OG, 128, 1).transpose(1, 0, 2).copy()

    hh = np.arange(HH, dtype=np.float32)
    ww = np.arange(W, dtype=np.float32)
    hio = np.broadcast_to(np.repeat(hh, W)[None, :], (KK, NHW)).astype(BF16)
    wio = np.broadcast_to(np.tile(ww, HH)[None, :], (KK, NHW)).astype(BF16)

    shared = dict(w_om=w_om, b_om=b_om, w_mm=w_mm, b_o=b_o, hio=hio, wio=wio)

    in_maps = []
    for core in range(NCORES):
        b, half = core // 2, core % 2
        h0 = half * HH
        xp = np.zeros((C, HP, WP), np.float32)
        glo, ghi = h0 - PAD, h0 + HH + PAD
        slo, shi = max(glo, 0), min(ghi, H)
        xp[:, slo - glo: slo - glo + (shi - slo), PAD:PAD + W] = x[b, :, slo:shi, :]
        xbf = xp.astype(BF16)
        x_cm = np.ascontiguousarray(xbf.reshape(CG, 128, HP, WP))
        xf = xbf.reshape(C, L)
        xTp = np.zeros((L, 2 * C), BF16)
        xTp[:, 0:C] = xf.T
        xTp[:L - 1, C:2 * C] = xf.T[1:]
        im = dict(shared)
        im["x_cm"] = x_cm
        im["xTp"] = xTp
        in_maps.append(im)
    return in_maps


_NC_CACHE = {}


def kernel(**inputs):
    if "nc" not in _NC_CACHE:
        _NC_CACHE["nc"] = build_nc()
    nc = _NC_CACHE["nc"]
    in_maps = host_prep(**inputs)
    res = bass_utils.run_bass_kernel_spmd(nc, in_maps,
                                          core_ids=list(range(NCORES)))
    out = np.zeros((B, O, H, W), np.float32)
    for core in range(NCORES):
        b, half = core // 2, core % 2
        yv = np.asarray(res.results[core]["y"], np.float32).reshape(O, HH, W)
        out[b, :, half * HH:(half + 1) * HH, :] = yv
    return out



# revision 5
# speedup vs baseline: 1.3187x; 1.3187x over previous
# DCNv2 (modulated deformable conv) Trainium2 Bass kernel.
#
# Sharding: pure data parallel over 8 cores; core = (batch, H-half), each
# core computes a (256, 32, 64) output slab from a zero-padded input slab.
#
# Per-core pipeline:
#   1. offset/mask 3x3 conv on the PE (bf16 matmuls, fp32 PSUM, fused
#      bias (+tap/pad constants) and sigmoid on the ACT engine)
#   2. sampling fields (py/px/floor/frac/bilinear corner weights incl.
#      mask) on DVE; weights written sigma-reordered in bf16
#   3. gather: SWDGE dma_gather from an HBM-resident 4-corner row table
#      (row l = [x[l], x[l+1], x[l+WP], x[l+WP+1]], 2KB) - ONE descriptor
#      per (tap, position), half the baseline's descriptor-gen cost
#   4. 4-corner weighted combine as 3 batched bf16 DVE ops per tile
#      (broadcast-AP weight multiply + strided pairwise adds)
#   5. colsT -> cols via a single XBAR DMA transpose per tile (frees the
#      PE and ACT engines)
#   6. main contraction out[o,hw] = sum_{c,p} w[o,c,p] cols[c,p,hw] as
#      bf16 matmuls accumulating in PSUM; bias + un-sigma on the ACT copy.
import numpy as np
import ml_dtypes

import concourse.bass as bass
import concourse.mybir as mybir
from concourse import bacc
import concourse.tile as tile
from concourse import library_config
from concourse.masks import make_identity
from concourse import bass_utils

BF16 = ml_dtypes.bfloat16

B, C, H, W = 4, 256, 64, 64
O, K = 256, 3
KK = K * K
NCORES = 8
HH = H // 2            # 32 output rows per core
PAD = 5                # zero halo; requires |offset| < PAD - 1
HP, WP = 48, 80        # padded local input dims
L = HP * WP            # 3840 source locations
NHW = HH * W           # 2048 output positions per core
NT = NHW // 128        # 16 gather tiles
CG = C // 128
OG = O // 128
A = mybir.AluOpType
ACTF = mybir.ActivationFunctionType
FP32 = mybir.dt.float32
BF = mybir.dt.bfloat16
I16 = mybir.dt.int16

MAGIC = float(np.float32(2 ** 23))


def build_nc():
    nc = bacc.Bacc("TRN2", target_bir_lowering=False, num_devices=NCORES)

    x_cm_d = nc.dram_tensor("x_cm", [CG, 128, HP, WP], BF, kind="ExternalInput").ap()
    xT4_d = nc.dram_tensor("xT4", [L, 4 * C], BF, kind="ExternalInput").ap()
    w_om_d = nc.dram_tensor("w_om", [128, KK, CG, 73], BF, kind="ExternalInput").ap()
    b_om_d = nc.dram_tensor("b_om", [73, 1], FP32, kind="ExternalInput").ap()
    w_mm_d = nc.dram_tensor("w_mm", [128, KK, CG, OG, 128], BF,
                            kind="ExternalInput").ap()
    b_o_d = nc.dram_tensor("b_o", [128, OG, 1], FP32, kind="ExternalInput").ap()
    hio_d = nc.dram_tensor("hio", [KK, NHW], BF, kind="ExternalInput").ap()
    wio_d = nc.dram_tensor("wio", [KK, NHW], BF, kind="ExternalInput").ap()
    y_d = nc.dram_tensor("y", [OG, 128, NHW], BF, kind="ExternalOutput").ap()

    with tile.TileContext(nc) as tc:
        with (
            tc.tile_pool(name="const", bufs=1) as const,
            tc.tile_pool(name="persist", bufs=1) as persist,
            tc.tile_pool(name="dramp", bufs=1, space="DRAM") as dramp,
            tc.tile_pool(name="ps_conv", bufs=1, space="PSUM") as ps_conv,
            tc.tile_pool(name="ps_cw", bufs=4, space="PSUM") as ps_cw,
            tc.tile_pool(name="ps_m", bufs=2, space="PSUM") as ps_m,
        ):
            # ---- constants into SBUF ----
            w_om = const.tile([128, KK, CG, 73], BF)
            nc.sync.dma_start(w_om[:], w_om_d)
            w_mm = const.tile([128, KK, CG, OG, 128], BF)
            nc.scalar.dma_start(w_mm[:], w_mm_d)
            b_om = const.tile([73, 1], FP32)
            nc.sync.dma_start(b_om[:], b_om_d)
            b_o = const.tile([128, OG, 1], FP32)
            nc.sync.dma_start(b_o[:], b_o_d)
            idbf = const.tile([128, 128], BF)
            make_identity(nc, idbf[:])
            nc.gpsimd.load_library(library_config.mlp)

            idx16 = persist.tile([KK, NHW], I16)
            # bilinear corner weights, sigma-reordered, bf16
            w00s = persist.tile([KK, NHW], BF)
            w01s = persist.tile([KK, NHW], BF)
            w10s = persist.tile([KK, NHW], BF)
            w11s = persist.tile([KK, NHW], BF)
            wrapped = persist.tile([128, NT, 72], I16)

            fld_cm = tc.tile_pool(name="fld", bufs=1)
            fld = fld_cm.__enter__()
            hio = fld.tile([KK, NHW], BF)
            nc.sync.dma_start(hio[:], hio_d)
            wio = fld.tile([KK, NHW], BF)
            nc.sync.dma_start(wio[:], wio_d)
            x_sb = []
            for cg in range(CG):
                xt = fld.tile([128, HP, WP], BF, name=f"xsb{cg}")
                # split the 737KB slab load across both HWDGE queues
                for i, eng in enumerate((nc.sync, nc.scalar, nc.sync)):
                    eng.dma_start(xt[:, i * 16:(i + 1) * 16], x_cm_d[cg][:, i * 16:(i + 1) * 16])
                x_sb.append(xt)

            # ---- offset/mask conv ----
            # psum channel layout: [0:9] off_y, [32:41] off_x, [64:73] mask
            # (engine APs may only start at partitions 0/32/64/96); each is
            # copied to its own base-0 tile (TensorTensor requires equal
            # base partitions for SBUF operands)
            offy_s = fld.tile([KK, NHW], FP32)
            offx_s = fld.tile([KK, NHW], FP32)
            msk = fld.tile([KK, NHW], FP32)
            for blk in range(8):
                # matmul rhs must be one contiguous free dim: stream 4 full
                # padded rows (N=320) and discard the pad columns on copy-out
                ps = ps_conv.tile([73, 4 * WP], FP32, tag="psc")
                r0 = blk * 4
                n = 0
                for cg in range(CG):
                    for tap in range(KK):
                        ky, kx = tap // K, tap % K
                        rhs = x_sb[cg][:, r0 + 4 + ky, 4 + kx:]
                        rhs = bass.AP(tensor=rhs.tensor, offset=rhs.offset,
                                      ap=[rhs.ap[0], [1, 4 * WP]])
                        nc.tensor.matmul(ps[:], w_om[:, tap, cg], rhs,
                                         start=(n == 0), stop=(n == 2 * KK - 1))
                        n += 1
                sl = slice(blk * 4 * W, (blk + 1) * 4 * W)
                psv = [None, None, None]
                for i, base in enumerate((0, 32, 64)):
                    p4 = ps[base:base + 9].rearrange("c (r x) -> c r x", r=4)
                    psv[i] = p4[:, :, 0:W]
                nc.scalar.activation(offy_s[:, sl], psv[0], ACTF.Identity,
                                     bias=b_om[0:9])
                nc.scalar.activation(offx_s[:, sl], psv[1], ACTF.Identity,
                                     bias=b_om[32:41])
                nc.scalar.activation(msk[:, sl], psv[2], ACTF.Sigmoid,
                                     bias=b_om[64:73])

            # ---- sampling fields [KK, NHW] f32 ----
            py = fld.tile([KK, NHW], FP32, tag="t_pyx", bufs=2, name="py")
            px = fld.tile([KK, NHW], FP32, tag="t_pyx", bufs=2, name="px")
            nc.vector.tensor_tensor(py[:], offy_s[:], hio[:], A.add)
            nc.vector.tensor_tensor(px[:], offx_s[:], wio[:], A.add)

            def floor_clamp(dst, src, hi):
                # dst = clamp(floor(src), 0, hi) via magic-number round(src-0.5)
                t1 = fld.tile([KK, NHW], FP32, tag="fc1", name="fc1", bufs=1)
                nc.vector.tensor_scalar(t1[:], src[:], MAGIC - 0.5, None, A.add)
                nc.vector.tensor_scalar(t1[:], t1[:], MAGIC, None, A.subtract)
                nc.vector.tensor_scalar(dst[:], t1[:], 0.0, float(hi),
                                        A.max, A.min)

            y0 = fld.tile([KK, NHW], FP32)
            x0 = fld.tile([KK, NHW], FP32)
            floor_clamp(y0, py, HP - 2)
            floor_clamp(x0, px, WP - 2)
            fy = fld.tile([KK, NHW], FP32)
            fx = fld.tile([KK, NHW], FP32)
            nc.vector.tensor_tensor(fy[:], py[:], y0[:], A.subtract)
            nc.vector.tensor_tensor(fx[:], px[:], x0[:], A.subtract)

            u = fld.tile([KK, NHW], FP32, tag="t_pyx", bufs=2, name="u")
            gy = fld.tile([KK, NHW], FP32, tag="t_pyx", bufs=2, name="gy")
            nc.vector.tensor_tensor(u[:], fy[:], msk[:], A.mult)
            nc.vector.tensor_tensor(gy[:], msk[:], u[:], A.subtract)

            # sigma-reordering view: natural position (t, p, a) is served by
            # partition a*16+p, so write weight fields at t*128 + a*16 + p
            def sig(wt):
                return bass.AP(tensor=wt.tensor, offset=wt.offset,
                               ap=[wt[:].ap[0], [128, NT], [1, 16], [16, 8]])

            def nat(ft):
                return ft[:].rearrange("c (t p a) -> c t p a", t=NT, p=16)

            # w01 = gy*fx ; w00 = gy - w01 ; w11 = u*fx ; w10 = u - w11
            # (all stores sigma-strided, bf16)
            nc.vector.tensor_tensor(sig(w01s), nat(gy), nat(fx), A.mult)
            nc.vector.tensor_tensor(sig(w00s), nat(gy), sig(w01s), A.subtract)
            nc.vector.tensor_tensor(sig(w11s), nat(u), nat(fx), A.mult)
            nc.vector.tensor_tensor(sig(w10s), nat(u), sig(w11s), A.subtract)

            idxf = fld.tile([KK, NHW], FP32, tag="idxf", name="idxf")
            nc.vector.scalar_tensor_tensor(idxf[:], y0[:], float(WP), x0[:],
                                           A.mult, A.add)
            nc.vector.tensor_copy(idx16[:], idxf[:])

            # idx -> DRAM bounce (partition-major [16, NT, 72]) -> replicated
            db = dramp.tile([16, NT, 72], I16, name="db")
            for t in range(NT):
                dst_ap = bass.AP(tensor=db.tensor, offset=db.offset + t * 72,
                                 ap=[[8, KK], [NT * 72, 16], [1, 8]])
                src_w = idx16[:, t * 128:(t + 1) * 128].rearrange(
                    "b (p q) -> b p q", p=16)
                nc.sync.dma_start(dst_ap, src_w)
            rep_ap = bass.AP(tensor=db.tensor, offset=db.offset,
                             ap=[[0, 8], [NT * 72, 16], [1, NT * 72]])
            nc.sync.dma_start(wrapped[:].rearrange("p t q -> p (t q)"), rep_ap)

            fld_cm.__exit__(None, None, None)

            # ---- per-tile gather + combine + matmul ----
            cwp_cm = tc.tile_pool(name="cwp", bufs=3)
            cwp = cwp_cm.__enter__()
            qp_cm = tc.tile_pool(name="qp", bufs=3)
            qp = qp_cm.__enter__()
            hp_cm = tc.tile_pool(name="hp", bufs=2)
            hp = hp_cm.__enter__()
            ctp_cm = tc.tile_pool(name="ctp", bufs=3)
            ctp = ctp_cm.__enter__()
            colsp_cm = tc.tile_pool(name="colsp", bufs=2)
            colsp = colsp_cm.__enter__()
            outp_cm = tc.tile_pool(name="outp", bufs=2)
            outp = outp_cm.__enter__()
            corners = [w00s, w01s, w10s, w11s]
            cols_sb = None
            for t in range(NT):
                # per-position corner weights -> cw4T[pos, tap, corner] bf16
                cw4T = cwp.tile([128, KK, 4], BF, tag="cw4T", name="cw4T")
                for j, wf in enumerate(corners):
                    psf = ps_cw.tile([128, KK], BF, tag="psf", name="psf")
                    nc.tensor.transpose(psf[:], wf[0:9, t * 128:(t + 1) * 128],
                                        idbf[0:9, 0:9])
                    dst = bass.AP(tensor=cw4T.tensor, offset=cw4T.offset + j,
                                  ap=[cw4T[:].ap[0], [4, KK]])
                    nc.scalar.activation(dst, psf[:], ACTF.Identity)

                # gather Q4[128(sig hw), 9(tap), 1024(4 corners x 256c)]
                Q4 = qp.tile([128, KK, 4 * C], BF, tag="Q4", name="Q4")
                nc.gpsimd.dma_gather(
                    out_ap=Q4[:], in_ap=xT4_d, idxs_ap=wrapped[:, t],
                    num_idxs=KK * 128, num_idxs_reg=KK * 128,
                    elem_size=4 * C, single_packet=False,
                )

                # weighted 4-corner combine: 3 batched bf16 DVE ops
                p0 = Q4[:].ap[0]
                q4d = bass.AP(tensor=Q4.tensor, offset=Q4.offset,
                              ap=[p0, [4 * C, KK], [C, 4], [1, C]])
                wbc = bass.AP(tensor=cw4T.tensor, offset=cw4T.offset,
                              ap=[cw4T[:].ap[0], [4, KK], [1, 4], [0, C]])
                nc.vector.tensor_tensor(q4d, q4d, wbc, A.mult)

                h = hp.tile([128, KK, 2, C], BF, tag="h", name="h")
                qe = bass.AP(tensor=Q4.tensor, offset=Q4.offset,
                             ap=[p0, [4 * C, KK], [2 * C, 2], [1, C]])
                qo = bass.AP(tensor=Q4.tensor, offset=Q4.offset + C,
                             ap=[p0, [4 * C, KK], [2 * C, 2], [1, C]])
                nc.vector.tensor_tensor(h[:], qe, qo, A.add)

                colsT = ctp.tile([128, KK, C], BF, tag="colsT", name="colsT")
                he = bass.AP(tensor=h.tensor, offset=h.offset,
                             ap=[h[:].ap[0], [2 * C, KK], [1, C]])
                ho = bass.AP(tensor=h.tensor, offset=h.offset + C,
                             ap=[h[:].ap[0], [2 * C, KK], [1, C]])
                nc.vector.tensor_tensor(colsT[:], he, ho, A.add)

                # transpose to cols [128(c), KK, CG, 512(sig hw)] via XBAR DMA
                if t % 4 == 0:
                    cols_sb = colsp.tile([128, KK, CG, 512], BF, tag="cols",
                                         name="cols")
                csrc = bass.AP(tensor=colsT.tensor, offset=colsT.offset,
                               ap=[colsT[:].ap[0], [1, KK * C]])
                cdst = bass.AP(tensor=cols_sb.tensor,
                               offset=cols_sb.offset + (t % 4) * 128,
                               ap=[cols_sb[:].ap[0], [512, KK * CG], [1, 128]])
                nc.sync.dma_start(cdst, csrc, transpose=True)

                # main contraction per 4-tile group
                if t % 4 == 3:
                    g = t // 4
                    for og in range(OG):
                        psO = ps_m.tile([128, 512], FP32, tag="psO", name="psO")
                        n = 0
                        for cg in range(CG):
                            for tap in range(KK):
                                nc.tensor.matmul(
                                    psO[:], w_mm[:, tap, cg, og],
                                    cols_sb[:, tap, cg],
                                    start=(n == 0), stop=(n == 2 * KK - 1))
                                n += 1
                        out_t = outp.tile([128, 512], BF, tag="out",
                                          name="out_t")
                        dst = out_t.rearrange(
                            "o (q p a) -> o q p a", q=4, p=16).transpose(
                            [0, 1, 3, 2])
                        nc.scalar.activation(dst, psO[:], ACTF.Identity,
                                             bias=b_o[:, og])
                        nc.scalar.dma_start(y_d[og, :, g * 512:(g + 1) * 512],
                                            out_t[:])
            for p in (outp_cm, colsp_cm, ctp_cm, hp_cm, qp_cm, cwp_cm):
                p.__exit__(None, None, None)
    nc.compile()
    return nc


# ---------------- host side ----------------

def host_prep(x, w_off, b_off, w_mask, b_mask, w_dcn, b_dcn):
    """Build the 8 per-core input maps (pure layout prep / sharding)."""
    x = np.asarray(x, np.float32)
    w_off = np.asarray(w_off, np.float32)
    w_mask = np.asarray(w_mask, np.float32)
    b_off = np.asarray(b_off, np.float32)
    b_mask = np.asarray(b_mask, np.float32)
    w_dcn = np.asarray(w_dcn, np.float32)
    b_dcn = np.asarray(b_dcn, np.float32)

    wcat = np.zeros((73, C, K, K), np.float32)
    wcat[0:9] = w_off[0::2]
    wcat[32:41] = w_off[1::2]
    wcat[64:73] = w_mask
    w_om = np.zeros((128, KK, CG, 73), BF16)
    for tap in range(KK):
        ky, kx = tap // K, tap % K
        for cg in range(CG):
            w_om[:, tap, cg] = (
                wcat[:, cg * 128:(cg + 1) * 128, ky, kx].T.astype(BF16))

    ky_t = np.repeat(np.arange(K), K).astype(np.float32)
    kx_t = np.tile(np.arange(K), K).astype(np.float32)
    b_om = np.zeros((73, 1), np.float32)
    b_om[0:9, 0] = b_off[0::2] + ky_t - 1 + PAD
    b_om[32:41, 0] = b_off[1::2] + kx_t - 1 + PAD
    b_om[64:73, 0] = b_mask

    w_mm = np.zeros((128, KK, CG, OG, 128), BF16)
    for tap in range(KK):
        ky, kx = tap // K, tap % K
        for cg in range(CG):
            for og in range(OG):
                w_mm[:, tap, cg, og] = w_dcn[
                    og * 128:(og + 1) * 128, cg * 128:(cg + 1) * 128,
                    ky, kx].T.astype(BF16)
    b_o = b_dcn.reshape(OG, 128, 1).transpose(1, 0, 2).copy()

    hh = np.arange(HH, dtype=np.float32)
    ww = np.arange(W, dtype=np.float32)
    hio = np.broadcast_to(np.repeat(hh, W)[None, :], (KK, NHW)).astype(BF16)
    wio = np.broadcast_to(np.tile(ww, HH)[None, :], (KK, NHW)).astype(BF16)

    shared = dict(w_om=w_om, b_om=b_om, w_mm=w_mm, b_o=b_o, hio=hio, wio=wio)

    in_maps = []
    for core in range(NCORES):
        b, half = core // 2, core % 2
        h0 = half * HH
        xp = np.zeros((C, HP, WP), np.float32)
        glo, ghi = h0 - PAD, h0 + HH + PAD
        slo, shi = max(glo, 0), min(ghi, H)
        xp[:, slo - glo: slo - glo + (shi - slo), PAD:PAD + W] = x[b, :, slo:shi, :]
        xbf = xp.astype(BF16)
        x_cm = np.ascontiguousarray(xbf.reshape(CG, 128, HP, WP))
        xT = np.ascontiguousarray(xbf.reshape(C, L).T)  # [L, C]
        xT4 = np.zeros((L, 4 * C), BF16)
        xT4[:, 0:C] = xT
        xT4[:L - 1, C:2 * C] = xT[1:]
        xT4[:L - WP, 2 * C:3 * C] = xT[WP:]
        xT4[:L - WP - 1, 3 * C:4 * C] = xT[WP + 1:]
        im = dict(shared)
        im["x_cm"] = x_cm
        im["xT4"] = xT4
        in_maps.append(im)
    return in_maps


_NC_CACHE = {}


def kernel(**inputs):
    if "nc" not in _NC_CACHE:
        _NC_CACHE["nc"] = build_nc()
    nc = _NC_CACHE["nc"]
    in_maps = host_prep(**inputs)
    res = bass_utils.run_bass_kernel_spmd(nc, in_maps,
                                          core_ids=list(range(NCORES)))
    out = np.zeros((B, O, H, W), np.float32)
    for core in range(NCORES):
        b, half = core // 2, core % 2
        yv = np.asarray(res.results[core]["y"], np.float32).reshape(O, HH, W)
        out[b, :, half * HH:(half + 1) * HH, :] = yv
    return out


# revision 14
# speedup vs baseline: 1.4830x; 1.1245x over previous
# DCNv2 (modulated deformable conv) Trainium2 Bass kernel.
#
# Sharding: pure data parallel over 8 cores; core = (batch, H-half), each
# core computes a (256, 32, 64) output slab from a zero-padded input slab.
#
# Per-core pipeline:
#   1. offset/mask 3x3 conv on the PE (bf16 matmuls, fp32 PSUM, fused
#      bias (+tap/pad constants) and sigmoid on the ACT engine)
#   2. sampling fields (py/px/floor/frac/bilinear corner weights incl.
#      mask) on DVE; weights written sigma-reordered in bf16
#   3. gather: SWDGE dma_gather from an HBM-resident 4-corner row table
#      (row l = [x[l], x[l+1], x[l+WP], x[l+WP+1]], 2KB) - ONE descriptor
#      per (tap, position), half the baseline's descriptor-gen cost
#   4. 4-corner weighted combine as 3 batched bf16 DVE ops per tile
#      (broadcast-AP weight multiply + strided pairwise adds)
#   5. colsT -> cols via a single XBAR DMA transpose per tile (frees the
#      PE and ACT engines)
#   6. main contraction out[o,hw] = sum_{c,p} w[o,c,p] cols[c,p,hw] as
#      bf16 matmuls accumulating in PSUM; bias + un-sigma on the ACT copy.
import numpy as np
import ml_dtypes

import concourse.bass as bass
import concourse.mybir as mybir
from concourse import bacc
import concourse.tile as tile
from concourse import library_config
from concourse.masks import make_identity
from concourse import bass_utils

BF16 = ml_dtypes.bfloat16

B, C, H, W = 4, 256, 64, 64
O, K = 256, 3
KK = K * K
NCORES = 8
HH = H // 2            # 32 output rows per core
PAD = 5                # zero halo; requires |offset| < PAD - 1
HP, WP = 48, 80        # padded local input dims
L = HP * WP            # 3840 source locations
NHW = HH * W           # 2048 output positions per core
NT = NHW // 128        # 16 gather tiles
CG = C // 128
OG = O // 128
A = mybir.AluOpType
ACTF = mybir.ActivationFunctionType
FP32 = mybir.dt.float32
BF = mybir.dt.bfloat16
I16 = mybir.dt.int16

MAGIC = float(np.float32(2 ** 23))


def build_nc():
    nc = bacc.Bacc("TRN2", target_bir_lowering=False, num_devices=NCORES)

    x_cm_d = nc.dram_tensor("x_cm", [CG, 128, HP, WP], BF, kind="ExternalInput").ap()
    xT4_d = nc.dram_tensor("xT4", [L, 4 * C], BF, kind="ExternalInput").ap()
    w_om_d = nc.dram_tensor("w_om", [128, KK, CG, 73], BF, kind="ExternalInput").ap()
    b_om_d = nc.dram_tensor("b_om", [73, 1], FP32, kind="ExternalInput").ap()
    w_mm_d = nc.dram_tensor("w_mm", [128, KK, CG, OG, 128], BF,
                            kind="ExternalInput").ap()
    b_o_d = nc.dram_tensor("b_o", [128, OG, 1], FP32, kind="ExternalInput").ap()
    hio_d = nc.dram_tensor("hio", [KK, NHW], BF, kind="ExternalInput").ap()
    wio_d = nc.dram_tensor("wio", [KK, NHW], BF, kind="ExternalInput").ap()
    y_d = nc.dram_tensor("y", [OG, 128, NHW], BF, kind="ExternalOutput").ap()

    with tile.TileContext(nc) as tc:
        with (
            tc.tile_pool(name="const", bufs=1) as const,
            tc.tile_pool(name="persist", bufs=1) as persist,
            tc.tile_pool(name="dramp", bufs=1, space="DRAM") as dramp,
            tc.tile_pool(name="ps_conv", bufs=1, space="PSUM") as ps_conv,
            tc.tile_pool(name="ps_cw", bufs=4, space="PSUM") as ps_cw,
            tc.tile_pool(name="ps_m", bufs=2, space="PSUM") as ps_m,
        ):
            # ---- constants into SBUF ----
            w_om = const.tile([128, KK, CG, 73], BF)
            nc.sync.dma_start(w_om[:], w_om_d)
            w_mm = const.tile([128, KK, CG, OG, 128], BF)
            nc.scalar.dma_start(w_mm[:], w_mm_d)
            b_om = const.tile([73, 1], FP32)
            nc.sync.dma_start(b_om[:], b_om_d)
            b_o = const.tile([128, OG, 1], FP32)
            nc.sync.dma_start(b_o[:], b_o_d)
            idbf = const.tile([128, 128], BF)
            make_identity(nc, idbf[:])
            nc.gpsimd.load_library(library_config.mlp)

            idx16 = persist.tile([KK, NHW], I16)
            # bilinear corner weights, sigma-reordered, bf16
            w00s = persist.tile([KK, NHW], BF)
            w01s = persist.tile([KK, NHW], BF)
            w10s = persist.tile([KK, NHW], BF)
            w11s = persist.tile([KK, NHW], BF)
            wrapped = persist.tile([128, NT, 72], I16)

            fld_cm = tc.tile_pool(name="fld", bufs=1)
            fld = fld_cm.__enter__()
            hio = fld.tile([KK, NHW], BF)
            nc.sync.dma_start(hio[:], hio_d)
            wio = fld.tile([KK, NHW], BF)
            nc.sync.dma_start(wio[:], wio_d)
            x_sb = []
            for cg in range(CG):
                xt = fld.tile([128, HP, WP], BF, name=f"xsb{cg}")
                # split the 737KB slab load across both HWDGE queues
                for i, eng in enumerate((nc.sync, nc.scalar, nc.sync)):
                    eng.dma_start(xt[:, i * 16:(i + 1) * 16], x_cm_d[cg][:, i * 16:(i + 1) * 16])
                x_sb.append(xt)

            # ---- offset/mask conv ----
            # psum channel layout: [0:9] off_y, [32:41] off_x, [64:73] mask
            # (engine APs may only start at partitions 0/32/64/96); each is
            # copied to its own base-0 tile (TensorTensor requires equal
            # base partitions for SBUF operands)
            offy_s = fld.tile([KK, NHW], FP32)
            offx_s = fld.tile([KK, NHW], FP32)
            msk = fld.tile([KK, NHW], BF)
            for blk in range(8):
                # matmul rhs must be one contiguous free dim: stream 4 full
                # padded rows (N=320) and discard the pad columns on copy-out
                ps = ps_conv.tile([73, 4 * WP], FP32, tag="psc")
                r0 = blk * 4
                n = 0
                for cg in range(CG):
                    for tap in range(KK):
                        ky, kx = tap // K, tap % K
                        rhs = x_sb[cg][:, r0 + 4 + ky, 4 + kx:]
                        rhs = bass.AP(tensor=rhs.tensor, offset=rhs.offset,
                                      ap=[rhs.ap[0], [1, 4 * WP]])
                        nc.tensor.matmul(ps[:], w_om[:, tap, cg], rhs,
                                         start=(n == 0), stop=(n == 2 * KK - 1))
                        n += 1
                sl = slice(blk * 4 * W, (blk + 1) * 4 * W)
                psv = [None, None, None]
                for i, base in enumerate((0, 32, 64)):
                    p4 = ps[base:base + 9].rearrange("c (r x) -> c r x", r=4)
                    psv[i] = p4[:, :, 0:W]
                nc.scalar.activation(offy_s[:, sl], psv[0], ACTF.Identity,
                                     bias=b_om[0:9])
                nc.scalar.activation(offx_s[:, sl], psv[1], ACTF.Identity,
                                     bias=b_om[32:41])
                nc.scalar.activation(msk[:, sl], psv[2], ACTF.Sigmoid,
                                     bias=b_om[64:73])

            # ---- sampling fields [KK, NHW] f32 ----
            py = fld.tile([KK, NHW], FP32, tag="t_pyx", bufs=2, name="py")
            px = fld.tile([KK, NHW], FP32, tag="t_pyx", bufs=2, name="px")
            nc.vector.tensor_tensor(py[:], offy_s[:], hio[:], A.add)
            nc.vector.tensor_tensor(px[:], offx_s[:], wio[:], A.add)

            def floor_clamp(dst, src, hi):
                # dst = clamp(floor(src), 0, hi) via magic-number round(src-0.5)
                t1 = fld.tile([KK, NHW], FP32, tag="fc1", name="fc1", bufs=1)
                nc.vector.tensor_scalar(t1[:], src[:], MAGIC - 0.5, None, A.add)
                nc.vector.tensor_scalar(t1[:], t1[:], MAGIC, None, A.subtract)
                nc.vector.tensor_scalar(dst[:], t1[:], 0.0, float(hi),
                                        A.max, A.min)

            y0 = fld.tile([KK, NHW], BF)
            x0 = fld.tile([KK, NHW], BF)
            floor_clamp(y0, py, HP - 2)
            floor_clamp(x0, px, WP - 2)
            fy = fld.tile([KK, NHW], BF)
            fx = fld.tile([KK, NHW], BF)
            nc.vector.tensor_tensor(fy[:], py[:], y0[:], A.subtract)
            nc.vector.tensor_tensor(fx[:], px[:], x0[:], A.subtract)

            u = fld.tile([KK, NHW], BF, tag="t_pyx", bufs=2, name="u")
            gy = fld.tile([KK, NHW], BF, tag="t_pyx", bufs=2, name="gy")
            nc.vector.tensor_tensor(u[:], fy[:], msk[:], A.mult)
            nc.vector.tensor_tensor(gy[:], msk[:], u[:], A.subtract)

            # w01 = gy*fx ; w00 = gy - w01 ; w11 = u*fx ; w10 = u - w11
            # (natural order, bf16), then sigma-reorder for the per-tile
            # transposes ([9,128] slices must be contiguous on the PE)
            w01n = fld.tile([KK, NHW], BF, tag="t_wn", bufs=4, name="w01n")
            w11n = fld.tile([KK, NHW], BF, tag="t_wn", bufs=4, name="w11n")
            w00n = fld.tile([KK, NHW], BF, tag="t_wn", bufs=4, name="w00n")
            w10n = fld.tile([KK, NHW], BF, tag="t_wn", bufs=4, name="w10n")
            nc.vector.tensor_tensor(w01n[:], gy[:], fx[:], A.mult)
            nc.vector.tensor_tensor(w00n[:], gy[:], w01n[:], A.subtract)
            nc.vector.tensor_tensor(w11n[:], u[:], fx[:], A.mult)
            nc.vector.tensor_tensor(w10n[:], u[:], w11n[:], A.subtract)

            def sigma_copy(wdst, wsrc):
                src = wsrc[:].rearrange("c (t p a) -> c t p a", t=NT,
                                        p=16).transpose([0, 1, 3, 2])
                nc.vector.tensor_copy(wdst[:], src)

            sigma_copy(w00s, w00n)
            sigma_copy(w01s, w01n)
            sigma_copy(w10s, w10n)
            sigma_copy(w11s, w11n)

            nc.vector.scalar_tensor_tensor(idx16[:], y0[:], float(WP), x0[:],
                                           A.mult, A.add)

            # idx -> DRAM bounce (partition-major [16, NT, 72]) -> replicated
            db = dramp.tile([16, NT, 72], I16, name="db")
            for t in range(NT):
                dst_ap = bass.AP(tensor=db.tensor, offset=db.offset + t * 72,
                                 ap=[[8, KK], [NT * 72, 16], [1, 8]])
                src_w = idx16[:, t * 128:(t + 1) * 128].rearrange(
                    "b (p q) -> b p q", p=16)
                nc.sync.dma_start(dst_ap, src_w)
            rep_ap = bass.AP(tensor=db.tensor, offset=db.offset,
                             ap=[[0, 8], [NT * 72, 16], [1, NT * 72]])
            nc.sync.dma_start(wrapped[:].rearrange("p t q -> p (t q)"), rep_ap)

            fld_cm.__exit__(None, None, None)

            # ---- per-tile gather + combine + matmul ----
            cwp_cm = tc.tile_pool(name="cwp", bufs=3)
            cwp = cwp_cm.__enter__()
            qp_cm = tc.tile_pool(name="qp", bufs=3)
            qp = qp_cm.__enter__()
            hp_cm = tc.tile_pool(name="hp", bufs=2)
            hp = hp_cm.__enter__()
            ctp_cm = tc.tile_pool(name="ctp", bufs=3)
            ctp = ctp_cm.__enter__()
            colsp_cm = tc.tile_pool(name="colsp", bufs=2)
            colsp = colsp_cm.__enter__()
            outp_cm = tc.tile_pool(name="outp", bufs=2)
            outp = outp_cm.__enter__()
            corners = [w00s, w01s, w10s, w11s]
            cols_sb = None
            for t in range(NT):
                # per-position corner weights -> cw4T[pos, tap, corner] bf16
                # (sigma permutation folded into the transpose's free walk:
                # out partition a*16+p reads natural column p*8+a)
                cw4T = cwp.tile([128, KK, 4], BF, tag="cw4T", name="cw4T")
                for j, wf in enumerate(corners):
                    psf = ps_cw.tile([128, KK], BF, tag="psf", name="psf")
                    nc.tensor.transpose(psf[:], wf[0:9, t * 128:(t + 1) * 128],
                                        idbf[0:9, 0:9])
                    dst = bass.AP(tensor=cw4T.tensor, offset=cw4T.offset + j,
                                  ap=[cw4T[:].ap[0], [4, KK]])
                    nc.scalar.activation(dst, psf[:], ACTF.Identity)

                # gather Q4[128(sig hw), 9(tap), 1024(4 corners x 256c)]
                Q4 = qp.tile([128, KK, 4 * C], BF, tag="Q4", name="Q4")
                nc.gpsimd.dma_gather(
                    out_ap=Q4[:], in_ap=xT4_d, idxs_ap=wrapped[:, t],
                    num_idxs=KK * 128, num_idxs_reg=KK * 128,
                    elem_size=4 * C, single_packet=False,
                )

                # weighted 4-corner combine: 3 batched bf16 DVE ops
                p0 = Q4[:].ap[0]
                q4d = bass.AP(tensor=Q4.tensor, offset=Q4.offset,
                              ap=[p0, [C, 4 * KK], [1, C]])
                wbc = bass.AP(tensor=cw4T.tensor, offset=cw4T.offset,
                              ap=[cw4T[:].ap[0], [1, 4 * KK], [0, C]])
                nc.vector.tensor_tensor(q4d, q4d, wbc, A.mult)

                h = hp.tile([128, KK, 2, C], BF, tag="h", name="h")
                qe = bass.AP(tensor=Q4.tensor, offset=Q4.offset,
                             ap=[p0, [4 * C, KK], [2 * C, 2], [1, C]])
                qo = bass.AP(tensor=Q4.tensor, offset=Q4.offset + C,
                             ap=[p0, [4 * C, KK], [2 * C, 2], [1, C]])
                nc.vector.tensor_tensor(h[:], qe, qo, A.add)

                colsT = ctp.tile([128, KK, C], BF, tag="colsT", name="colsT")
                he = bass.AP(tensor=h.tensor, offset=h.offset,
                             ap=[h[:].ap[0], [2 * C, KK], [1, C]])
                ho = bass.AP(tensor=h.tensor, offset=h.offset + C,
                             ap=[h[:].ap[0], [2 * C, KK], [1, C]])
                nc.vector.tensor_tensor(colsT[:], he, ho, A.add)

                # transpose to cols [128(c), KK, CG, 512(sig hw)] via XBAR DMA
                if t % 4 == 0:
                    cols_sb = colsp.tile([128, KK, CG, 512], BF, tag="cols",
                                         name="cols")
                csrc = bass.AP(tensor=colsT.tensor, offset=colsT.offset,
                               ap=[colsT[:].ap[0], [1, KK * C]])
                cdst = bass.AP(tensor=cols_sb.tensor,
                               offset=cols_sb.offset + (t % 4) * 128,
                               ap=[cols_sb[:].ap[0], [512, KK * CG], [1, 128]])
                nc.sync.dma_start(cdst, csrc, transpose=True)

                # main contraction per 4-tile group
                if t % 4 == 3:
                    g = t // 4
                    for og in range(OG):
                        psO = ps_m.tile([128, 512], FP32, tag="psO", name="psO")
                        n = 0
                        for cg in range(CG):
                            for tap in range(KK):
                                nc.tensor.matmul(
                                    psO[:], w_mm[:, tap, cg, og],
                                    cols_sb[:, tap, cg],
                                    start=(n == 0), stop=(n == 2 * KK - 1))
                                n += 1
                        out_t = outp.tile([128, 512], BF, tag="out",
                                          name="out_t")
                        dst = out_t.rearrange(
                            "o (q p a) -> o q p a", q=4, p=16).transpose(
                            [0, 1, 3, 2])
                        nc.scalar.activation(dst, psO[:], ACTF.Identity,
                                             bias=b_o[:, og])
                        nc.scalar.dma_start(y_d[og, :, g * 512:(g + 1) * 512],
                                            out_t[:])
            for p in (outp_cm, colsp_cm, ctp_cm, hp_cm, qp_cm, cwp_cm):
                p.__exit__(None, None, None)
    nc.compile()
    return nc


# ---------------- host side ----------------

def host_prep(x, w_off, b_off, w_mask, b_mask, w_dcn, b_dcn):
    """Build the 8 per-core input maps (pure layout prep / sharding)."""
    x = np.asarray(x, np.float32)
    w_off = np.asarray(w_off, np.float32)
    w_mask = np.asarray(w_mask, np.float32)
    b_off = np.asarray(b_off, np.float32)
    b_mask = np.asarray(b_mask, np.float32)
    w_dcn = np.asarray(w_dcn, np.float32)
    b_dcn = np.asarray(b_dcn, np.float32)

    wcat = np.zeros((73, C, K, K), np.float32)
    wcat[0:9] = w_off[0::2]
    wcat[32:41] = w_off[1::2]
    wcat[64:73] = w_mask
    w_om = np.zeros((128, KK, CG, 73), BF16)
    for tap in range(KK):
        ky, kx = tap // K, tap % K
        for cg in range(CG):
            w_om[:, tap, cg] = (
                wcat[:, cg * 128:(cg + 1) * 128, ky, kx].T.astype(BF16))

    ky_t = np.repeat(np.arange(K), K).astype(np.float32)
    kx_t = np.tile(np.arange(K), K).astype(np.float32)
    b_om = np.zeros((73, 1), np.float32)
    b_om[0:9, 0] = b_off[0::2] + ky_t - 1 + PAD
    b_om[32:41, 0] = b_off[1::2] + kx_t - 1 + PAD
    b_om[64:73, 0] = b_mask

    w_mm = np.zeros((128, KK, CG, OG, 128), BF16)
    for tap in range(KK):
        ky, kx = tap // K, tap % K
        for cg in range(CG):
            for og in range(OG):
                w_mm[:, tap, cg, og] = w_dcn[
                    og * 128:(og + 1) * 128, cg * 128:(cg + 1) * 128,
                    ky, kx].T.astype(BF16)
    b_o = b_dcn.reshape(OG, 128, 1).transpose(1, 0, 2).copy()

    hh = np.arange(HH, dtype=np.float32)
    ww = np.arange(W, dtype=np.float32)
    hio = np.broadcast_to(np.repeat(hh, W)[None, :], (KK, NHW)).astype(BF16)
    wio = np.broadcast_to(np.tile(ww, HH)[None, :], (KK, NHW)).astype(BF16)

    shared = dict(w_om=w_om, b_om=b_om, w_mm=w_mm, b_o=b_o, hio=hio, wio=wio)

    in_maps = []
    for core in range(NCORES):
        b, half = core // 2, core % 2
        h0 = half * HH
        xp = np.zeros((C, HP, WP), np.float32)
        glo, ghi = h0 - PAD, h0 + HH + PAD
        slo, shi = max(glo, 0), min(ghi, H)
        xp[:, slo - glo: slo - glo + (shi - slo), PAD:PAD + W] = x[b, :, slo:shi, :]
        xbf = xp.astype(BF16)
        x_cm = np.ascontiguousarray(xbf.reshape(CG, 128, HP, WP))
        xT = np.ascontiguousarray(xbf.reshape(C, L).T)  # [L, C]
        xT4 = np.zeros((L, 4 * C), BF16)
        xT4[:, 0:C] = xT
        xT4[:L - 1, C:2 * C] = xT[1:]
        xT4[:L - WP, 2 * C:3 * C] = xT[WP:]
        xT4[:L - WP - 1, 3 * C:4 * C] = xT[WP + 1:]
        im = dict(shared)
        im["x_cm"] = x_cm
        im["xT4"] = xT4
        in_maps.append(im)
    return in_maps


_NC_CACHE = {}


def kernel(**inputs):
    if "nc" not in _NC_CACHE:
        _NC_CACHE["nc"] = build_nc()
    nc = _NC_CACHE["nc"]
    in_maps = host_prep(**inputs)
    res = bass_utils.run_bass_kernel_spmd(nc, in_maps,
                                          core_ids=list(range(NCORES)))
    out = np.zeros((B, O, H, W), np.float32)
    for core in range(NCORES):
        b, half = core // 2, core % 2
        yv = np.asarray(res.results[core]["y"], np.float32).reshape(O, HH, W)
        out[b, :, half * HH:(half + 1) * HH, :] = yv
    return out
